# revision 6
# baseline (speedup 1.0000x reference)
"""Trainium2 Bass kernel for a single-layer MHA decode step with KV cache.

Problem (hardcoded from spec):
  x            [32, 8, 2048]      query tokens (B=32 batches x T=8 steps)
  cache_keys   [32, 32, 1016, 64] (B, H, S_cache, Dh)
  cache_values [32, 32, 1016, 64]
  Wq/Wk/Wv/Wo  [2048, 2048], biases [2048]
  out = MHA(x, cache) @ Wo.T + bo   -> [32, 8, 2048]

Sharding: tensor-parallel over heads. Each of the 8 cores handles 4 heads:
QKV projections for its head slice, attention over its KV-cache slice, and a
partial output projection (rank-256 slice of Wo). Host sums the 8 partials.

Per-core layout:
 - x and weights pre-transposed on host so matmuls see contraction on
   partitions.  Wq/Wk rows permuted to (dhalf m, head h, j) so scores pack
   4 heads x 32 contraction rows into one 128-partition matmul (accumulating
   over m).  Score rows land as (bb, h, t) = 4x4x8 = 128 PSUM partitions.
 - Key/score axis permuted by sigma(j) = 8*(j%128) + j//128, baked into kT on
   host.  AV contraction chunk c is then the stride-8 set {8p + c}, so V
   streams from natural layout with 2 KiB per-partition DMA rows.  Virtual
   s in [1016, 1024) = freshly projected K/V: new-K scores go through a
   separate psum (columns 127 mod 128 of the attn tile), new V lands on
   partition 127 of the V tile via a small DRAM scratch round-trip.
"""

import numpy as np

import concourse.bass as bass
import concourse.mybir as mybir
import concourse.tile as tile
from concourse import bacc
from concourse import bass_utils
from concourse.masks import make_identity

F32 = mybir.dt.float32
F32R = mybir.dt.float32r
BF16 = mybir.dt.bfloat16

B, T, D = 32, 8, 2048
H, DH = 32, 64
S_CACHE, S = 1016, 1024
N_CORES = 8
HC = H // N_CORES          # heads per core = 4
TOK = B * T                # 256
QD = HC * DH               # 256 per-core qkv dims
N_ROUNDS = 8               # 4 batches per round
BB = 4                     # batches per round

AF = mybir.ActivationFunctionType
ALU = mybir.AluOpType
AX = mybir.AxisListType

# dtype knobs: "f32" (exact) or "f32r" (tf32-like, 4x faster for FD>=256)
CFG = {
    "proj": "f32",     # QKV + Wo projection matmuls
    "scores": "f32",   # Q @ K^T
    "av": "f32",       # attn @ V
}


def build_nc(cfg=CFG):
    nc = bacc.Bacc(None, target_bir_lowering=False)
    sc_dt = F32R if cfg["scores"] == "f32r" else F32
    av_dt = F32R if cfg["av"] == "f32r" else F32
    pj_dt = F32R if cfg["proj"] == "f32r" else F32

    xT = nc.dram_tensor("xT", [128, 16, 256], pj_dt, kind="ExternalInput")
    wqT = nc.dram_tensor("wqT", [128, 16, 256], pj_dt, kind="ExternalInput")
    wkT = nc.dram_tensor("wkT", [128, 16, 256], pj_dt, kind="ExternalInput")
    wvT = nc.dram_tensor("wvT", [128, 16, 256], pj_dt, kind="ExternalInput")
    woT = nc.dram_tensor("woT", [128, 2, 2048], pj_dt, kind="ExternalInput")
    bq = nc.dram_tensor("bq", [256], F32, kind="ExternalInput")
    bk = nc.dram_tensor("bk", [256], F32, kind="ExternalInput")
    bv = nc.dram_tensor("bv", [256], F32, kind="ExternalInput")
    bo = nc.dram_tensor("bo", [2048], F32, kind="ExternalInput")
    # kT[b, m, q=(h,j), jcol]: sigma-permuted key columns (zeros at new-key cols)
    kT = nc.dram_tensor("kT", [B, 2, 128, S], sc_dt, kind="ExternalInput")
    # v natural layout [b, h, s_cache, dh]
    v = nc.dram_tensor("v", [B, HC, S_CACHE, DH], av_dt, kind="ExternalInput")
    out = nc.dram_tensor("out", [TOK, D], F32, kind="ExternalOutput")
    # flat scratch: [m, p=(b_local, t), (h, dh)] mirrors the vnew SBUF tiles
    vnew_scratch = nc.dram_tensor("vnew_scratch", [2, 128, 256], av_dt,
                                  kind="Internal")

    with tile.TileContext(nc) as tc:
        with (
            tc.tile_pool(name="singles", bufs=1) as singles,
            tc.tile_pool(name="stream", bufs=10) as stream,
            tc.tile_pool(name="attn_pool", bufs=2) as attn_pool,
            tc.tile_pool(name="small", bufs=8) as small,
            tc.tile_pool(name="ps_scores", bufs=2, space="PSUM") as ps_scores,
            tc.tile_pool(name="ps_transp", bufs=1, space="PSUM") as ps_transp,
            tc.tile_pool(name="ps_univ", bufs=2, space="PSUM") as ps_univ,
        ):
            # ---- persistent tiles ----
            xT_sb = singles.tile([128, 16, 256], pj_dt)
            wq_sb = singles.tile([128, 16, 256], pj_dt)
            wk_sb = singles.tile([128, 16, 256], pj_dt)
            wv_sb = singles.tile([128, 16, 256], pj_dt)
            wo_sb = singles.tile([128, 2, 2048], pj_dt)
            nc.sync.dma_start(xT_sb, xT[:, :, :])
            nc.sync.dma_start(wq_sb, wqT[:, :, :])
            nc.sync.dma_start(wk_sb, wkT[:, :, :])
            nc.sync.dma_start(wv_sb, wvT[:, :, :])
            nc.sync.dma_start(wo_sb, woT[:, :, :])

            bq_sb = singles.tile([128, 2], F32)
            bk_sb = singles.tile([128, 2], F32)
            nc.sync.dma_start(bq_sb, bq[:].rearrange("(m p) -> p m", p=128))
            nc.sync.dma_start(bk_sb, bk[:].rearrange("(m p) -> p m", p=128))
            bv_bc = singles.tile([128, 256], F32)
            nc.sync.dma_start(
                bv_bc, bass.AP(tensor=bv[:].tensor, offset=0, ap=[[0, 128], [1, 256]])
            )
            bo_bc = singles.tile([128, 2048], F32)
            nc.sync.dma_start(
                bo_bc, bass.AP(tensor=bo[:].tensor, offset=0, ap=[[0, 128], [1, 2048]])
            )

            ident = singles.tile([128, 128], F32)
            make_identity(nc, ident)

            # Q in block-diag layout: qbd[32h+j, m, 32b + 8h + t]
            qbd = singles.tile([128, 2, 1024], sc_dt)
            nc.vector.memset(qbd, 0.0)
            knew = singles.tile([128, 2, 256], sc_dt)  # [q=(h,j), m, (b,t)]
            # attnout^T accumulated: [ao-half p, a, tok]
            aoT = singles.tile([128, 2, 256], pj_dt)

            # ---- projections ----
            for m in range(2):
                psq = ps_univ.tile([128, 512], F32, name=f"psq_{m}", tag="u")[:, :256]
                psk = ps_univ.tile([128, 512], F32, name=f"psk_{m}", tag="u")[:, :256]
                for k in range(16):
                    st = dict(start=(k == 0), stop=(k == 15))
                    nc.tensor.matmul(
                        psq, wq_sb[:, k, 128 * m:128 * m + 128],
                        xT_sb[:, k, :], **st)
                for k in range(16):
                    st = dict(start=(k == 0), stop=(k == 15))
                    nc.tensor.matmul(
                        psk, wk_sb[:, k, 128 * m:128 * m + 128],
                        xT_sb[:, k, :], **st)
                # evac Q into block-diag (strided) + bias; psum rows 32h+j
                for h in range(4):
                    rows = slice(32 * h, 32 * h + 32)
                    out_ap = qbd[rows, m, :].rearrange("p (b w) -> p b w", w=32)[
                        :, :, 8 * h:8 * h + 8
                    ]
                    in_ap = psq[rows, :].rearrange("p (b t) -> p b t", t=8)
                    nc.scalar.activation(out_ap, in_ap, AF.Identity,
                                         bias=bq_sb[rows, m:m + 1], scale=1.0)
                nc.scalar.activation(knew[:, m, :], psk, AF.Identity,
                                     bias=bk_sb[:, m:m + 1], scale=1.0)

            for m in range(2):
                psv = ps_univ.tile([128, 512], F32, name=f"psv_{m}", tag="u")[:, :256]
                for k in range(16):
                    st = dict(start=(k == 0), stop=(k == 15))
                    nc.tensor.matmul(
                        psv, xT_sb[:, k, 128 * m:128 * m + 128],
                        wv_sb[:, k, :], **st)
                vnew_sb = small.tile([128, 256], av_dt, name=f"vnew_sb_{m}",
                                     tag="vnew", bufs=2)
                nc.vector.tensor_add(vnew_sb, psv, bv_bc)
                nc.sync.dma_start(vnew_scratch[m, :, :], vnew_sb)

            # ---- attention rounds ----
            for r in range(N_ROUNDS):
                pss = ps_scores.tile([128, 1024], F32, name="pss", tag="pss")
                psn = ps_univ.tile([128, 512], F32, name="psn", tag="u")[:, :8]
                for bb in range(BB):
                    b = BB * r + bb
                    orow = slice(32 * bb, 32 * bb + 32)
                    kt00 = stream.tile([128, 512], sc_dt, name="kt00", tag="kt")
                    kt10 = stream.tile([128, 512], sc_dt, name="kt10", tag="kt")
                    kt01 = stream.tile([128, 512], sc_dt, name="kt01", tag="kt")
                    kt11 = stream.tile([128, 512], sc_dt, name="kt11", tag="kt")
                    nc.sync.dma_start(kt00, kT[b, 0, :, 0:512])
                    nc.sync.dma_start(kt10, kT[b, 1, :, 0:512])
                    nc.sync.dma_start(kt01, kT[b, 0, :, 512:1024])
                    nc.sync.dma_start(kt11, kT[b, 1, :, 512:1024])
                    for m, kta, ktb in ((0, kt00, kt01), (1, kt10, kt11)):
                        lhsT = qbd[:, m, 32 * b:32 * b + 32]
                        st = dict(start=(m == 0), stop=(m == 1))
                        tp = (0, 32 * bb)
                        nc.tensor.matmul(pss[orow, 0:512], lhsT, kta,
                                         tile_position=tp, **st)
                        nc.tensor.matmul(pss[orow, 512:1024], lhsT, ktb,
                                         tile_position=tp, **st)
                        nc.tensor.matmul(psn[orow, :], lhsT,
                                         knew[:, m, 8 * b:8 * b + 8],
                                         tile_position=tp, **st)

                # softmax over 1024+8 cols; cache part excludes cols 127 mod 128
                cache_ap = pss.rearrange("p (c w) -> p c w", w=128)[:, :, 0:127]
                nmx = small.tile([128, 1], F32, name="nmx", tag="sm1")
                nmx2 = small.tile([128, 1], F32, name="nmx2", tag="sm1")
                nc.vector.reduce_max(nmx, cache_ap, axis=AX.XY, negate=True)
                nc.vector.reduce_max(nmx2, psn, axis=AX.X, negate=True)
                # nmx/nmx2 hold -max  ->  combined -max = min
                nc.vector.tensor_tensor(nmx, nmx, nmx2, ALU.min)
                nbias = small.tile([128, 1], F32, name="nbias", tag="sm1")
                nc.vector.tensor_scalar_mul(nbias, nmx, 0.125)

                attn = attn_pool.tile([128, 1024], F32, name="attn", tag="attn")
                s1 = small.tile([128, 1], F32, name="s1", tag="sm1")
                s2 = small.tile([128, 1], F32, name="s2", tag="sm1")
                attn3 = attn.rearrange("p (c w) -> p c w", w=128)
                nc.scalar.activation(attn3[:, :, 0:127], cache_ap, AF.Exp,
                                     bias=nbias, scale=0.125, accum_out=s1)
                nc.scalar.activation(attn3[:, :, 127:128],
                                     psn.rearrange("p (c w) -> p c w", w=1),
                                     AF.Exp, bias=nbias, scale=0.125, accum_out=s2)
                nc.vector.tensor_add(s1, s1, s2)
                recip = small.tile([128, 1], F32, name="recip", tag="sm1")
                nc.vector.reciprocal(recip, s1)
                nc.vector.tensor_scalar_mul(attn, attn, recip)

                # transpose attn -> attnT [s-local, (bb,h,t)] (cast for AV dtype)
                attnT = attn_pool.tile([128, 8, 128], av_dt, name="attnT", tag="attnT")
                pst = ps_transp.tile([128, 8, 128], F32, name="pst", tag="pst")
                for c in range(8):
                    nc.tensor.transpose(pst[:, c, :], attn[:, 128 * c:128 * c + 128],
                                        ident)
                nc.scalar.copy(attnT, pst)

                # AV: psav[32bb + t, 64h + dh]
                psav = ps_univ.tile([128, 512], F32, name="psav", tag="u")[:, :256]
                nc.vector.memset(psav, 0.0)
                for bb in range(BB):
                    b = BB * r + bb
                    for h in range(HC):
                        vt = stream.tile([128, 8, 64], av_dt, name="vt", tag="vt")
                        nc.sync.dma_start(
                            vt[0:127, :, :],
                            v[b, h, :, :].rearrange("(p i) d -> p i d", i=8))
                        nc.sync.dma_start(
                            vt[127:128, :, :],
                            vnew_scratch[b // 16, 8 * (b % 16):8 * (b % 16) + 8,
                                         64 * h:64 * h + 64][None])
                        for c in range(8):
                            nc.tensor.matmul(
                                psav[32 * bb:32 * bb + 8, 64 * h:64 * h + 64],
                                attnT[:, c, 32 * bb + 8 * h:32 * bb + 8 * h + 8],
                                vt[:, c, :], tile_position=(0, 32 * bb),
                                start=(c == 0), stop=(c == 7))

                # evac attnout, transpose to [ao, tok], compact into aoT
                ao_sb = small.tile([128, 256], F32, name="ao_sb", tag="ao", bufs=2)
                nc.scalar.copy(ao_sb, psav)
                for a in range(2):
                    psu = ps_univ.tile([128, 512], F32, name=f"psu_{a}",
                                       tag="u")[:, :128]
                    nc.tensor.transpose(psu, ao_sb[:, 128 * a:128 * a + 128], ident)
                    in_ap = psu.rearrange("p (b w) -> p b w", w=32)[:, :, 0:8]
                    out_ap = aoT[:, a, 32 * r:32 * r + 32].rearrange(
                        "p (b t) -> p b t", t=8)
                    nc.vector.tensor_copy(out_ap, in_ap)

            # ---- output projection (partial over this core's 256 ao dims) ----
            for mt in range(2):
                for ob in range(4):
                    pso = ps_univ.tile([128, 512], F32, name=f"pso_{mt}_{ob}",
                                       tag="u")
                    for a in range(2):
                        nc.tensor.matmul(
                            pso, aoT[:, a, 128 * mt:128 * mt + 128],
                            wo_sb[:, a, 512 * ob:512 * ob + 512],
                            start=(a == 0), stop=(a == 1))
                    osb = small.tile([128, 512], F32, name=f"osb_{mt}_{ob}",
                                     tag="osb", bufs=2)
                    nc.vector.tensor_add(osb, pso, bo_bc[:, 512 * ob:512 * ob + 512])
                    nc.sync.dma_start(
                        out[128 * mt:128 * mt + 128, 512 * ob:512 * ob + 512], osb)

    nc.finalize()
    return nc


_SIGMA = None


def _sigma():
    # sigma(j) = virtual key index at score column j
    global _SIGMA
    if _SIGMA is None:
        j = np.arange(S)
        _SIGMA = 8 * (j % 128) + j // 128
    return _SIGMA


def _prep_core(c, x_flat_T, cache_keys, cache_values, Wq, bq, Wk, bk, Wv, bv, Wo, bo):
    hs = slice(HC * c, HC * c + HC)
    qs = slice(QD * c, QD * c + QD)

    def perm_rows(W):
        # rows ordered (m, h, j): row 32h + j of tile m = W[64h + 32m + j]
        Ws = W[qs].reshape(HC, 2, 32, -1)              # [h, m, j, d]
        return Ws.transpose(1, 0, 2, 3).reshape(QD, -1)  # [(m,h,j), d]

    wq_p = perm_rows(Wq)
    wk_p = perm_rows(Wk)
    bq_p = np.ascontiguousarray(perm_rows(bq[:, None])[:, 0])
    bk_p = np.ascontiguousarray(perm_rows(bk[:, None])[:, 0])

    def as_tiles(WT):  # [D, 256] -> [128, 16, 256]
        return np.ascontiguousarray(WT.reshape(16, 128, QD).transpose(1, 0, 2))

    wqT = as_tiles(np.ascontiguousarray(wq_p.T))
    wkT = as_tiles(np.ascontiguousarray(wk_p.T))
    wvT = as_tiles(np.ascontiguousarray(Wv[qs].T))
    woT = np.ascontiguousarray(
        Wo[:, qs].T.reshape(2, 128, D).transpose(1, 0, 2))   # [128, 2, 2048]

    # kT[b, m, (h,j), jcol]: keys sigma-permuted; zero at new-key columns
    ck = cache_keys[:, hs]                        # [B, 4, 1016, 64]
    kmat = ck.reshape(B, HC, S_CACHE, 2, 32).transpose(0, 3, 1, 4, 2)  # b m h j s
    kmat = np.ascontiguousarray(kmat.reshape(B, 2, 128, S_CACHE))
    kT = np.zeros((B, 2, 128, S), dtype=np.float32)
    sig = _sigma()
    valid = sig < S_CACHE
    kT[:, :, :, valid] = kmat[:, :, :, sig[valid]]

    return {
        "xT": x_flat_T,
        "wqT": wqT, "wkT": wkT, "wvT": wvT, "woT": woT,
        "bq": bq_p, "bk": bk_p,
        "bv": np.ascontiguousarray(bv[qs]),
        "bo": bo,
        "kT": kT,
        "v": np.ascontiguousarray(cache_values[:, hs]),
    }


_NC_CACHE = {}


def kernel(x, cache_keys, cache_values, Wq, bq, Wk, bk, Wv, bv, Wo, bo):
    x = np.asarray(x, dtype=np.float32)
    cache_keys = np.asarray(cache_keys, dtype=np.float32)
    cache_values = np.asarray(cache_values, dtype=np.float32)
    Wq, Wk, Wv, Wo = (np.asarray(w, dtype=np.float32) for w in (Wq, Wk, Wv, Wo))
    bq, bk, bv, bo = (np.asarray(b_, dtype=np.float32) for b_ in (bq, bk, bv, bo))

    x_flat_T = np.ascontiguousarray(
        x.reshape(TOK, D).T.reshape(16, 128, TOK).transpose(1, 0, 2))  # [128,16,256]

    in_maps = [
        _prep_core(c, x_flat_T, cache_keys, cache_values,
                   Wq, bq, Wk, bk, Wv, bv, Wo, bo)
        for c in range(N_CORES)
    ]

    key = tuple(sorted(CFG.items()))
    if key not in _NC_CACHE:
        _NC_CACHE[key] = build_nc(CFG)
    nc = _NC_CACHE[key]

    res = bass_utils.run_bass_kernel_spmd(nc, in_maps, core_ids=list(range(N_CORES)))
    out = np.zeros((TOK, D), dtype=np.float32)
    for r in res.results:
        out += r["out"]
    return out.reshape(B, T, D)
